# revision 7
# baseline (speedup 1.0000x reference)
"""BiLSTM-CRF loss kernel for 8 Trainium2 NeuronCores.

Sharding: data-parallel over batch (B=128 -> b=16/core), weights replicated.

Per-core pipeline:
  A) input projections pre = W_ih @ x + bias, both directions, fp32 matmuls,
     psum evacuated by DVE with bias folded in; emission interleaved with (B).
  B) 512 sequential LSTM steps, both directions as independent latency chains.
     Gates layout [128 part, 8 chunk x 16 b].  Single Sigmoid table:
     tanh(x) = 2*sigmoid(2x)-1, with the 2x folded into host-prepped weights
     and h stored as h_tilde = h/2 (2x folded into W_hh / w_emit), so a step is
     1 sigmoid call + one sigmoid(2c) call + fused scalar_tensor_tensor ops.
  C) emissions E = w_emit'.T @ h2 + b_emit; gold emission score via host-built
     one-hot mask (label preprocessing only) with fused mul-accumulate; gold
     transition score via host-built transition-count matrix vs the
     device-resident transition table; exp(E) in place.
  D) CRF forward in exp domain: a <- expE_t * (expT^T a) * r, a real 64x64
     matmul per step (exp(T) host-precomputed) with delayed mean
     normalization; per-step normalizers batch-ln'd once at the end.
     loss = ln(sum a) - sum ln r - gold_emit - gold_trans.
"""

import numpy as np
import ml_dtypes

B, L, E, H, K = 128, 512, 512, 256, 64
NCORES = 8
b = B // NCORES          # 16
G1 = 4 * H               # 1024
G = 2 * G1               # 2048
H2 = 2 * H               # 512
BANK = 512               # psum bank free size (fp32)
LBANK = BANK // b        # 32 l-positions per bank
HRING = 8                # steps per h-tilde DMA chunk

_BF16 = ml_dtypes.bfloat16
_built = {}


def _build(Lc, dbg=False):
    """Build the SPMD Bass program for sequence length Lc (Lc % 64 == 0)."""
    from contextlib import ExitStack

    import concourse.mybir as mybir
    import concourse.tile as tile
    from concourse import bacc

    dt = mybir.dt
    f32, bf16 = dt.float32, dt.bfloat16
    AF = mybir.ActivationFunctionType
    OP = mybir.AluOpType

    NB = Lc // LBANK
    assert Lc % 64 == 0 and NB >= 2

    nc = bacc.Bacc(
        "TRN2",
        target_bir_lowering=False,
        debug=False,
        enable_asserts=False,
        num_devices=NCORES,
    )

    xT = nc.dram_tensor("xT", [E, Lc * b], f32, kind="ExternalInput").ap()
    wihT = nc.dram_tensor("wihT", [E, G], f32, kind="ExternalInput").ap()
    whhT = nc.dram_tensor("whhT", [H, G], bf16, kind="ExternalInput").ap()
    biasG = nc.dram_tensor("biasG", [G], f32, kind="ExternalInput").ap()
    wemT = nc.dram_tensor("wemT", [H2, K], bf16, kind="ExternalInput").ap()
    bem = nc.dram_tensor("bem", [K], f32, kind="ExternalInput").ap()
    expTM = nc.dram_tensor("expTM", [K, 2 * K], f32, kind="ExternalInput").ap()
    onehot = nc.dram_tensor("onehot", [K, Lc * b], bf16, kind="ExternalInput").ap()
    ctneg = nc.dram_tensor("ctneg", [K * K, b], f32, kind="ExternalInput").ap()
    tflat = nc.dram_tensor("tflat", [K * K], f32, kind="ExternalInput").ap()
    loss = nc.dram_tensor("loss", [b], f32, kind="ExternalOutput").ap()
    hkind = "ExternalOutput" if dbg else "Internal"
    hdram = [
        nc.dram_tensor("hscr_f", [128, 2, Lc, b], bf16, kind=hkind).ap(),
        nc.dram_tensor("hscr_b", [128, 2, Lc, b], bf16, kind=hkind).ap(),
    ]
    if dbg:
        dbg_E = nc.dram_tensor("dbg_E", [K, Lc * b], f32, kind="ExternalOutput").ap()
        dbg_expE = nc.dram_tensor("dbg_expE", [K, Lc * b], f32, kind="ExternalOutput").ap()
        dbg_rbuf = nc.dram_tensor("dbg_rbuf", [1, Lc * b], f32, kind="ExternalOutput").ap()
        dbg_a = nc.dram_tensor("dbg_a", [K, b], f32, kind="ExternalOutput").ap()
        dbg_ge = nc.dram_tensor("dbg_ge", [K, b], f32, kind="ExternalOutput").ap()

    with tile.TileContext(nc) as tc, ExitStack() as top:
        singles = top.enter_context(tc.tile_pool(name="singles", bufs=1))

        wih_sb = singles.tile([128, 4, G], f32)
        nc.sync.dma_start(out=wih_sb, in_=wihT.rearrange("(kc p) g -> p kc g", p=128))
        whh_sb = singles.tile([128, 2, G], bf16)
        nc.sync.dma_start(out=whh_sb, in_=whhT.rearrange("(kc p) g -> p kc g", p=128))
        bias_sb = singles.tile([128, G // 128], f32)
        nc.sync.dma_start(out=bias_sb, in_=biasG.rearrange("(m p) -> p m", p=128))
        wem_sb = singles.tile([128, 4, K], bf16)
        nc.sync.dma_start(out=wem_sb, in_=wemT.rearrange("(kc p) k -> p kc k", p=128))
        bem_sb = singles.tile([K, 1], f32)
        nc.sync.dma_start(out=bem_sb, in_=bem.unsqueeze(1))
        expT_sb = singles.tile([K, K], f32)
        nc.sync.dma_start(out=expT_sb, in_=expTM[:, 0:K])
        meanT_sb = singles.tile([K, K], f32)
        nc.sync.dma_start(out=meanT_sb, in_=expTM[:, K : 2 * K])
        tflat_sb = singles.tile([128, 32], f32)
        nc.sync.dma_start(out=tflat_sb, in_=tflat.rearrange("(c p) -> p c", p=128))
        ctneg_sb = singles.tile([128, 32, b], f32)
        nc.sync.dma_start(out=ctneg_sb, in_=ctneg.rearrange("(c p) b -> p c b", p=128))

        onesP_sb = singles.tile([K, 1], f32)
        nc.vector.memset(onesP_sb, 1.0)
        onesN_sb = singles.tile([K, 1], f32)
        nc.vector.memset(onesN_sb, -1.0)
        h0 = singles.tile([128, 2, b], bf16)
        nc.vector.memset(h0, 0.0)

        E_sb = singles.tile([K, Lc * b], f32)       # E, then expE in place
        rbuf = singles.tile([1, Lc * b], f32)       # CRF mean reciprocals
        nc.vector.memset(rbuf, 1.0)

        # ================= phases A + B (interleaved emission) ==============
        with ExitStack() as ab:
            xpool = ab.enter_context(tc.tile_pool(name="xring", bufs=1))
            psA = ab.enter_context(tc.tile_pool(name="psA", bufs=4, space="PSUM"))
            prepool = ab.enter_context(tc.tile_pool(name="pre", bufs=2))
            psB = ab.enter_context(tc.tile_pool(name="psB", bufs=2, space="PSUM"))
            sgpool = ab.enter_context(tc.tile_pool(name="sg", bufs=2))
            tmppool = ab.enter_context(tc.tile_pool(name="tmp", bufs=2))
            cpool = ab.enter_context(tc.tile_pool(name="cst", bufs=2))
            hring = ab.enter_context(tc.tile_pool(name="hring", bufs=2))

            pre_tiles = [[None] * NB, [None] * NB]
            c_prev = [None, None]
            hring_t = [None, None]

            def emit_bank_proj(dir_, nb):
                xt = xpool.tile([128, 4, BANK], f32, tag=f"x{dir_}")
                nc.sync.dma_start(
                    out=xt,
                    in_=xT.rearrange("(kc p) n -> p kc n", p=128)[
                        :, :, nb * BANK : (nb + 1) * BANK
                    ],
                )
                pt = prepool.tile([128, 8, LBANK, b], bf16, tag=f"pre{dir_}")
                pre_tiles[dir_][nb] = pt
                for mi8 in range(8):
                    mi = dir_ * 8 + mi8
                    ps = psA.tile([128, BANK], f32, tag="psA")
                    for kc in range(4):
                        nc.tensor.matmul(
                            ps,
                            lhsT=wih_sb[:, kc, mi * 128 : (mi + 1) * 128],
                            rhs=xt[:, kc, :],
                            start=(kc == 0),
                            stop=(kc == 3),
                        )
                    nc.vector.tensor_scalar_add(
                        pt[:, mi8].rearrange("p l c -> p (l c)"),
                        ps,
                        bias_sb[:, mi : mi + 1],
                    )

            def rslot(dir_, s):
                return s % HRING if dir_ == 0 else HRING - 1 - s % HRING

            def emit_step(s):
                for dir_ in (0, 1):
                    j = s // LBANK
                    nbank = j if dir_ == 0 else NB - 1 - j
                    lofs = s % LBANK if dir_ == 0 else LBANK - 1 - s % LBANK
                    if s == 0:
                        h_prev = h0
                    else:
                        h_prev = hring_t[dir_][:, :, rslot(dir_, s - 1), :]
                    ps = psB.tile([128, 8, b], f32, tag=f"g{dir_}")
                    for mi8 in range(8):
                        for kc in range(2):
                            nc.tensor.matmul(
                                ps[:, mi8, :],
                                lhsT=whh_sb[
                                    :, kc,
                                    dir_ * G1 + mi8 * 128 : dir_ * G1 + (mi8 + 1) * 128,
                                ],
                                rhs=h_prev[:, kc, :],
                                start=(kc == 0),
                                stop=(kc == 1),
                            )
                    nc.vector.tensor_add(
                        ps, ps, pre_tiles[dir_][nbank][:, :, lofs, :]
                    )
                    sg = sgpool.tile([128, 4, 2, b], f32, tag=f"sg{dir_}")
                    nc.scalar.activation(sg, ps.rearrange("p (g c) e -> p g c e", g=4), AF.Sigmoid)
                    t_ = tmppool.tile([128, 2, b], f32, tag=f"t{dir_}")
                    nc.vector.scalar_tensor_tensor(
                        out=t_, in0=sg[:, 3], scalar=0.5, in1=sg[:, 0],
                        op0=OP.subtract, op1=OP.mult,
                    )
                    cn = cpool.tile([128, 2, b], f32, tag=f"c{dir_}")
                    if s == 0:
                        nc.vector.tensor_scalar_mul(cn, t_, 2.0)
                    else:
                        v = tmppool.tile([128, 2, b], f32, tag=f"v{dir_}")
                        nc.vector.tensor_mul(v, sg[:, 1], c_prev[dir_])
                        nc.vector.scalar_tensor_tensor(
                            out=cn, in0=t_, scalar=2.0, in1=v,
                            op0=OP.mult, op1=OP.add,
                        )
                    c_prev[dir_] = cn
                    s2c = tmppool.tile([128, 2, b], f32, tag=f"s2c{dir_}")
                    nc.scalar.activation(s2c, cn, AF.Sigmoid, scale=2.0)
                    if s % HRING == 0:
                        hring_t[dir_] = hring.tile(
                            [128, 2, HRING, b], bf16, tag=f"h{dir_}", name=f"ht{dir_}"
                        )
                    nc.vector.scalar_tensor_tensor(
                        out=hring_t[dir_][:, :, rslot(dir_, s), :],
                        in0=s2c, scalar=0.5, in1=sg[:, 2],
                        op0=OP.subtract, op1=OP.mult,
                    )
                    if s % HRING == HRING - 1:
                        lpos = s - (HRING - 1) if dir_ == 0 else Lc - 1 - s
                        nc.sync.dma_start(
                            out=hdram[dir_][:, :, lpos : lpos + HRING, :],
                            in_=hring_t[dir_],
                        )

            for j in range(NB):
                emit_bank_proj(0, j)
                emit_bank_proj(1, NB - 1 - j)
                for s in range(j * LBANK, (j + 1) * LBANK):
                    emit_step(s)

        gsc = top.enter_context(tc.tile_pool(name="gsc", bufs=1))
        fin = top.enter_context(tc.tile_pool(name="fin", bufs=1))
        gered = gsc.tile([K, b], f32)
        gescr = gsc.tile([K, Lc], f32)
        E_v = E_sb.rearrange("k (l c) -> k l c", c=b)

        # ================= phase C: emissions + gold ========================
        with ExitStack() as pc:
            hin = pc.enter_context(tc.tile_pool(name="hin", bufs=2))
            psC = pc.enter_context(tc.tile_pool(name="psC", bufs=4, space="PSUM"))
            ohpool = pc.enter_context(tc.tile_pool(name="oh", bufs=2))

            for nb in range(NB):
                hf = hin.tile([128, 2, LBANK, b], bf16, tag="hf")
                nc.sync.dma_start(
                    out=hf, in_=hdram[0][:, :, nb * LBANK : (nb + 1) * LBANK, :]
                )
                hb = hin.tile([128, 2, LBANK, b], bf16, tag="hb")
                nc.sync.dma_start(
                    out=hb, in_=hdram[1][:, :, nb * LBANK : (nb + 1) * LBANK, :]
                )
                pse = psC.tile([K, BANK], f32, tag="psC")
                for kc4 in range(4):
                    rhs = (hf if kc4 < 2 else hb)[:, kc4 % 2]
                    nc.tensor.matmul(
                        pse,
                        lhsT=wem_sb[:, kc4, :],
                        rhs=rhs.rearrange("p l c -> p (l c)"),
                        start=(kc4 == 0),
                        stop=(kc4 == 3),
                    )
                nc.vector.tensor_scalar_add(
                    E_sb[:, nb * BANK : (nb + 1) * BANK], pse, bem_sb
                )

            if dbg:
                nc.sync.dma_start(out=dbg_E, in_=E_sb)
            for bb in range(b):
                oh = ohpool.tile([K, Lc], bf16, tag="oh")
                nc.sync.dma_start(
                    out=oh,
                    in_=onehot.rearrange("k (l c) -> k l c", c=b)[:, :, bb],
                )
                nc.vector.scalar_tensor_tensor(
                    out=gescr, in0=E_v[:, :, bb], scalar=1.0, in1=oh,
                    op0=OP.mult, op1=OP.mult,
                    accum_out=gered[:, bb : bb + 1],
                )
            for i in range(4):
                w = Lc * b // 4
                nc.scalar.activation(
                    E_sb[:, i * w : (i + 1) * w],
                    E_sb[:, i * w : (i + 1) * w],
                    AF.Exp,
                )

        if dbg:
            nc.sync.dma_start(out=dbg_expE, in_=E_sb)
        # ================= phase D: CRF scan ================================
        CH = 2
        bh = b // CH
        a_last = []
        with ExitStack() as pd:
            psD = pd.enter_context(tc.tile_pool(name="psD", bufs=2, space="PSUM"))
            apool = pd.enter_context(tc.tile_pool(name="aD", bufs=2))
            wpool = pd.enter_context(tc.tile_pool(name="wD", bufs=2))

            a_prev = [E_v[:, 0, h * bh : (h + 1) * bh] for h in range(CH)]
            for t in range(1, Lc):
                for h in range(CH):
                    col = t * b + h * bh
                    psM = psD.tile([K, bh], f32, tag=f"M{h}")
                    nc.tensor.matmul(psM, lhsT=meanT_sb, rhs=a_prev[h],
                                     start=True, stop=True)
                    psS = psD.tile([K, bh], f32, tag=f"S{h}")
                    nc.tensor.matmul(psS, lhsT=expT_sb, rhs=a_prev[h],
                                     start=True, stop=True)
                    # r_t = 1/Smean (current step, lane-aligned via replicated mean)
                    rt = wpool.tile([K, bh], f32, tag=f"r{h}")
                    nc.vector.reciprocal(rt, psM)
                    # record one row of r_t for the final ln-sum
                    nc.vector.tensor_copy(rbuf[:, col : col + bh], rt[0:1, :])
                    qt = wpool.tile([K, bh], f32, tag=f"q{h}")
                    nc.vector.tensor_mul(qt, E_sb[:, col : col + bh], rt)
                    an = apool.tile([K, bh], f32, tag=f"a{h}")
                    nc.vector.tensor_mul(an, psS, qt)
                    a_prev[h] = an
            afin = fin.tile([K, b], f32)
            for h in range(CH):
                nc.vector.tensor_copy(afin[:, h * bh : (h + 1) * bh], a_prev[h])
            if dbg:
                nc.sync.dma_start(out=dbg_rbuf, in_=rbuf)
                nc.sync.dma_start(out=dbg_a, in_=afin)
                nc.sync.dma_start(out=dbg_ge, in_=gered)

        # ================= final assembly ===================================
        with ExitStack() as pf:
            psF = pf.enter_context(tc.tile_pool(name="psF", bufs=1, space="PSUM"))
            psf1 = psF.tile([1, b], f32, tag="f1")
            nc.tensor.matmul(psf1, lhsT=onesP_sb, rhs=afin, start=True, stop=True)
            lnA = fin.tile([1, b], f32)
            nc.scalar.activation(lnA, psf1, AF.Ln)

            rlnbuf = fin.tile([1, Lc * b], f32)
            nc.scalar.activation(rlnbuf, rbuf, AF.Ln)
            lnr_sum = fin.tile([1, b], f32)
            nc.vector.tensor_reduce(
                lnr_sum, rlnbuf.rearrange("o (q c) -> o c q", c=b),
                axis=mybir.AxisListType.X, op=OP.add,
            )
            psf2 = psF.tile([1, b], f32, tag="f2")
            nc.tensor.matmul(psf2, lhsT=onesN_sb, rhs=gered, start=True, stop=False)
            for c in range(32):
                nc.tensor.matmul(
                    psf2,
                    lhsT=tflat_sb[:, c : c + 1],
                    rhs=ctneg_sb[:, c, :],
                    start=False,
                    stop=(c == 31),
                )
            tsum = fin.tile([1, b], f32)
            nc.vector.tensor_add(tsum, lnA, psf2)
            loss_sb = fin.tile([1, b], f32)
            nc.vector.tensor_sub(loss_sb, tsum, lnr_sum)
            nc.sync.dma_start(out=loss.unsqueeze(0), in_=loss_sb)

    nc.compile()
    return nc


def _reorder_gates(w):
    """PyTorch gate row order [i,f,g,o] -> device order [i,f,o,2g]."""
    i, f, g, o = np.split(w, 4, axis=0)
    return np.concatenate([i, f, o, 2.0 * g], axis=0)


def _host_prep(inputs, core, Lc):
    s = core * b
    x = np.asarray(inputs["input_batch"], np.float32)[s : s + b]       # (b,L,E)
    labels = np.asarray(inputs["labels"])[s : s + b].astype(np.int64)   # (b,L)

    xt = np.ascontiguousarray(np.transpose(x, (2, 1, 0))).reshape(E, Lc * b)

    wih = np.concatenate(
        [_reorder_gates(np.asarray(inputs["w_ih_f"], np.float32)),
         _reorder_gates(np.asarray(inputs["w_ih_b"], np.float32))], axis=0)
    whh = 2.0 * np.concatenate(
        [_reorder_gates(np.asarray(inputs["w_hh_f"], np.float32)),
         _reorder_gates(np.asarray(inputs["w_hh_b"], np.float32))], axis=0)
    bias = np.concatenate(
        [_reorder_gates((np.asarray(inputs["b_ih_f"], np.float32)
                         + np.asarray(inputs["b_hh_f"], np.float32))[:, None]),
         _reorder_gates((np.asarray(inputs["b_ih_b"], np.float32)
                         + np.asarray(inputs["b_hh_b"], np.float32))[:, None])],
        axis=0)[:, 0]

    wem = 2.0 * np.asarray(inputs["w_emit"], np.float32)               # (K, 2H)
    T = np.asarray(inputs["transition"], np.float32)
    expT = np.exp(T.astype(np.float64)).astype(np.float32)
    expTM = np.concatenate(
        [expT, np.repeat(expT.mean(axis=1, keepdims=True), K, axis=1)], axis=1)

    onehot = np.zeros((K, Lc, b), _BF16)
    ll = np.arange(Lc)[:, None].repeat(b, 1)
    bbx = np.arange(b)[None, :].repeat(Lc, 0)
    onehot[labels.T, ll, bbx] = 1

    flat = labels[:, :-1] * K + labels[:, 1:]
    Cmat = np.zeros((b, K * K), np.float32)
    for bb in range(b):
        np.add.at(Cmat[bb], flat[bb], 1.0)

    return {
        "xT": np.ascontiguousarray(xt),
        "wihT": np.ascontiguousarray(wih.T),
        "whhT": np.ascontiguousarray(whh.T.astype(_BF16)),
        "biasG": np.ascontiguousarray(bias),
        "wemT": np.ascontiguousarray(wem.T.astype(_BF16)),
        "bem": np.ascontiguousarray(np.asarray(inputs["b_emit"], np.float32)),
        "expTM": np.ascontiguousarray(expTM),
        "onehot": np.ascontiguousarray(onehot.reshape(K, Lc * b)),
        "ctneg": np.ascontiguousarray(-Cmat.T),
        "tflat": np.ascontiguousarray(T.reshape(K * K)),
    }


def kernel(**inputs) -> np.ndarray:
    from concourse import bass_utils

    if "nc" not in _built:
        _built["nc"] = _build(L)
    nc = _built["nc"]
    in_maps = [_host_prep(inputs, c, L) for c in range(NCORES)]
    res = bass_utils.run_bass_kernel_spmd(nc, in_maps, list(range(NCORES)))
    out = np.concatenate([np.asarray(res.results[c]["loss"]) for c in range(NCORES)])
    return out.astype(np.float32)
